# revision 3
# baseline (speedup 1.0000x reference)
"""Trainium2 Bass kernel for KinematicModel: 1-NN part labeling + LBS skinning.

Strategy (8 NeuronCores, query points sharded N/8 per core):
  - Host: forward kinematics (tiny, [32,16,4,4]) + feature packing.
  - Device per core, per 128-query block:
      PE:  scores s[q,m] = 2*x_q . c_m - |c_m|^2  via K=4 matmul,
           canos laid out label-interleaved (16 labels x 32 slots per
           512-chunk) so a strided DVE max-reduce yields per-label maxes.
      DVE: per-chunk [128,16,32] max-reduce -> per-label running max ->
           global max -> one-hot(label) -> label id.
      PE:  transpose of the one-hot [128,16]->[16,128] (accumulated to
           a [16, N] mask), then skinning as a K=64 matmul with
           weights W[(j,L),(t,i)] built from the FK transforms.
  - Host: gather shards, inverse layout, cast dtypes.

Self-contained: hardcodes shapes from the problem spec
(input_pc [50000,3], cano_pc [8192,3], seg_part [8192], theta [32,15]).
"""

import numpy as np
from contextlib import ExitStack

import concourse.bacc as bacc
import concourse.tile as tile
from concourse import mybir
from concourse.bass_utils import run_bass_kernel_spmd

F32 = mybir.dt.float32

N_CORES = 8
N_FULL = 50000
N_PER = N_FULL // N_CORES            # 6250
NBLK = (N_PER + 127) // 128          # 49
N_PAD = NBLK * 128                   # 6272
M = 8192
T = 32
E = 15
P = 16
EPS = 1e-8
NEG_BIG = -3.0e38

_CACHE = {}


# ----------------------------------------------------------------------------
# Host-side forward kinematics (mirrors reference.py in float32 numpy)
# ----------------------------------------------------------------------------

def _fk_host(axis_list, moment_list, theta_list):
    axis_list = axis_list.astype(np.float32)
    moment_list = moment_list.astype(np.float32)
    theta_list = theta_list.astype(np.float32)
    l = axis_list / (np.linalg.norm(axis_list, axis=-1, keepdims=True).astype(np.float32) + np.float32(EPS))
    q = np.cross(l, moment_list).astype(np.float32)
    x, y, z = l[:, 0], l[:, 1], l[:, 2]
    zero = np.zeros_like(x)
    K = np.stack([
        np.stack([zero, -z, y], -1),
        np.stack([z, zero, -x], -1),
        np.stack([-y, x, zero], -1),
    ], -2).astype(np.float32)
    KK = np.einsum('eij,ejk->eik', K, K).astype(np.float32)
    s = np.sin(theta_list)[:, :, None, None].astype(np.float32)
    c = np.cos(theta_list)[:, :, None, None].astype(np.float32)
    I3 = np.eye(3, dtype=np.float32)
    R = (I3 + s * K + (np.float32(1.0) - c) * KK).astype(np.float32)
    t = np.einsum('teij,ej->tei', I3 - R, q).astype(np.float32)
    T_, E_ = theta_list.shape
    G = np.zeros((T_, E_, 4, 4), np.float32)
    G[:, :, :3, :3] = R
    G[:, :, :3, 3] = t
    G[:, :, 3, 3] = 1.0
    W = np.broadcast_to(np.eye(4, dtype=np.float32), (T_, 4, 4)).copy()
    worlds = [W]
    for e in range(E_):
        W = np.matmul(W, G[:, e]).astype(np.float32)
        worlds.append(W)
    return np.stack(worlds, axis=1)          # [T, P, 4, 4]


# ----------------------------------------------------------------------------
# Device program
# ----------------------------------------------------------------------------

def _build_program(n_chunks):
    mcols = n_chunks * 512
    nc = bacc.Bacc('TRN2', target_bir_lowering=False, debug=False,
                   enable_asserts=False, num_devices=N_CORES)
    qf_d = nc.dram_tensor('qf', [4, N_PAD], F32, kind='ExternalInput').ap()
    qf16_d = nc.dram_tensor('qf16', [64, N_PAD], F32, kind='ExternalInput').ap()
    cf_d = nc.dram_tensor('cf', [4, mcols], F32, kind='ExternalInput').ap()
    wsk_d = nc.dram_tensor('wsk', [64, 96], F32, kind='ExternalInput').ap()
    iota_d = nc.dram_tensor('iota16', [128, 16], F32, kind='ExternalInput').ap()
    ident_d = nc.dram_tensor('ident', [128, 128], F32, kind='ExternalInput').ap()
    seg_d = nc.dram_tensor('seg_o', [128, NBLK], F32, kind='ExternalOutput').ap()
    pct_d = nc.dram_tensor('pct_o', [96, N_PAD], F32, kind='ExternalOutput').ap()

    with tile.TileContext(nc) as tc, ExitStack() as ctx:
        const = ctx.enter_context(tc.tile_pool(name='const', bufs=1))
        qf_s = const.tile([4, N_PAD], F32)
        qf16_s = const.tile([64, N_PAD], F32)
        cf_s = const.tile([4, mcols], F32)
        wsk_s = const.tile([64, 96], F32)
        iota_s = const.tile([128, 16], F32)
        ident_s = const.tile([128, 128], F32)
        ohT_s = const.tile([16, N_PAD], F32)
        segacc = const.tile([128, NBLK], F32)
        nc.sync.dma_start(qf_s[:], qf_d[:])
        nc.sync.dma_start(qf16_s[:], qf16_d[:])
        nc.sync.dma_start(cf_s[:], cf_d[:])
        nc.sync.dma_start(wsk_s[:], wsk_d[:])
        nc.sync.dma_start(iota_s[:], iota_d[:])
        nc.sync.dma_start(ident_s[:], ident_d[:])

        with tc.tile_pool(name='score_ps', bufs=6, space='PSUM') as score_ps, \
             tc.tile_pool(name='oh_ps', bufs=2, space='PSUM') as oh_ps, \
             tc.tile_pool(name='small', bufs=4) as small:
            for b in range(NBLK):
                lhs = qf_s[:, b * 128:(b + 1) * 128]
                labacc = small.tile([128, 16, n_chunks], F32, tag='labacc')
                for c in range(n_chunks):
                    ps = score_ps.tile([128, 512], F32)
                    nc.tensor.matmul(ps[:], lhs, cf_s[:, c * 512:(c + 1) * 512],
                                     start=True, stop=True)
                    nc.vector.tensor_reduce(
                        out=labacc[:, :, c],
                        in_=ps[:].rearrange('p (l s) -> p l s', l=16),
                        axis=mybir.AxisListType.X, op=mybir.AluOpType.max)
                lab16 = small.tile([128, 16], F32, tag='lab16')
                nc.vector.tensor_reduce(out=lab16[:], in_=labacc[:, :, :],
                                        axis=mybir.AxisListType.X,
                                        op=mybir.AluOpType.max)
                vmax = small.tile([128, 1], F32, tag='vmax')
                nc.vector.tensor_reduce(out=vmax[:], in_=lab16[:],
                                        axis=mybir.AxisListType.X,
                                        op=mybir.AluOpType.max)
                onehot = small.tile([128, 16], F32, tag='onehot')
                nc.vector.tensor_scalar(onehot[:], lab16[:], vmax[:], None,
                                        mybir.AluOpType.is_equal)
                trash = small.tile([128, 16], F32, tag='trash')
                nc.vector.tensor_mul(trash[:], onehot[:], iota_s[:])
                nc.vector.tensor_reduce(out=segacc[:, b:b + 1], in_=trash[:],
                                        axis=mybir.AxisListType.X,
                                        op=mybir.AluOpType.max)
                onehot2 = small.tile([128, 16], F32, tag='onehot2')
                nc.vector.tensor_scalar(onehot2[:], iota_s[:],
                                        segacc[:, b:b + 1], None,
                                        mybir.AluOpType.is_equal)
                ohp = oh_ps.tile([16, 128], F32)
                nc.tensor.transpose(ohp[:], onehot2[:], ident_s[:])
                nc.scalar.copy(out=ohT_s[:, b * 128:(b + 1) * 128], in_=ohp[:])

        nc.sync.dma_start(seg_d[:], segacc[:])

        # skinning: f[(j,L), n] = ohT[L, n] * qf16[(j,L), n]; pct = wsk.T @ f
        ohT4 = const.tile([64, N_PAD], F32)
        for j in range(4):
            nc.sync.dma_start(ohT4[16 * j:16 * (j + 1), :], ohT_s[:, :])
        nc.vector.tensor_mul(ohT4[:], ohT4[:], qf16_s[:])
        pout = const.tile([96, N_PAD], F32)
        with tc.tile_pool(name='skin_ps', bufs=4, space='PSUM') as skin_ps:
            nsk = N_PAD // 512            # 6272 = 12*512 + 128
            for c in range(nsk):
                pssk = skin_ps.tile([96, 512], F32)
                nc.tensor.matmul(pssk[:], wsk_s[:],
                                 ohT4[:, c * 512:(c + 1) * 512],
                                 start=True, stop=True)
                nc.scalar.copy(out=pout[:, c * 512:(c + 1) * 512], in_=pssk[:])
            rem = N_PAD - nsk * 512
            if rem:
                pssk = skin_ps.tile([96, 512], F32)
                nc.tensor.matmul(pssk[:, :rem], wsk_s[:], ohT4[:, nsk * 512:],
                                 start=True, stop=True)
                nc.scalar.copy(out=pout[:, nsk * 512:], in_=pssk[:, :rem])
        nc.sync.dma_start(pct_d[:], pout[:])

    nc.compile()
    return nc


# ----------------------------------------------------------------------------
# Entry point
# ----------------------------------------------------------------------------

def kernel(input_pc, cano_pc, seg_part, axis_list, moment_list, theta_list):
    input_pc = np.ascontiguousarray(np.asarray(input_pc, np.float32))
    cano_pc = np.ascontiguousarray(np.asarray(cano_pc, np.float32))
    seg_np = np.asarray(seg_part)
    labels = seg_np.astype(np.int64) % 16

    trans_list = _fk_host(np.asarray(axis_list, np.float32),
                          np.asarray(moment_list, np.float32),
                          np.asarray(theta_list, np.float32))

    # ---- cano features, label-interleaved: col = 512*c + 32*L + s ----
    counts = np.bincount(labels, minlength=16)
    n_chunks = int(np.ceil(counts.max() / 32))
    mcols = n_chunks * 512
    cf = np.zeros((4, mcols), np.float32)
    cf[3, :] = NEG_BIG
    cn2 = ((cano_pc[:, 0] * cano_pc[:, 0] + cano_pc[:, 1] * cano_pc[:, 1])
           + cano_pc[:, 2] * cano_pc[:, 2]).astype(np.float32)
    for L in range(16):
        idx = np.nonzero(labels == L)[0]
        k = np.arange(len(idx))
        cols = 512 * (k // 32) + 32 * L + (k % 32)
        cf[0, cols] = cano_pc[idx, 0]
        cf[1, cols] = cano_pc[idx, 1]
        cf[2, cols] = cano_pc[idx, 2]
        cf[3, cols] = -cn2[idx]

    # ---- skinning weights W[(j,L), (t,i)] ----
    R = trans_list[:, :, :3, :3]          # [T,P,3,3]
    tr = trans_list[:, :, :3, 3]          # [T,P,3]
    wsk = np.zeros((64, 96), np.float32)
    for j in range(4):
        for L in range(16):
            k = 16 * j + L
            if j < 3:
                wsk[k, :] = (R[:, L, :, j] * np.float32(0.5)).reshape(96)
            else:
                wsk[k, :] = tr[:, L, :].reshape(96)

    iota16 = np.broadcast_to(np.arange(16, dtype=np.float32), (128, 16)).copy()
    ident = np.eye(128, dtype=np.float32)

    key = n_chunks
    if key not in _CACHE:
        _CACHE[key] = _build_program(n_chunks)
    nc = _CACHE[key]

    in_maps = []
    for cidx in range(N_CORES):
        xs = input_pc[cidx * N_PER:(cidx + 1) * N_PER]
        qf = np.zeros((4, N_PAD), np.float32)
        qf[0, :N_PER] = 2.0 * xs[:, 0]
        qf[1, :N_PER] = 2.0 * xs[:, 1]
        qf[2, :N_PER] = 2.0 * xs[:, 2]
        qf[3, :N_PER] = 1.0
        qf16 = np.tile(qf[:, None, :], (1, 16, 1)).reshape(64, N_PAD)
        in_maps.append({'qf': qf, 'qf16': qf16, 'cf': cf, 'wsk': wsk,
                        'iota16': iota16, 'ident': ident})

    res = run_bass_kernel_spmd(nc, in_maps, list(range(N_CORES)))

    pc_parts, seg_parts_out = [], []
    for cidx in range(N_CORES):
        om = res.results[cidx]
        segf = om['seg_o']                       # [128, NBLK]
        seg_core = segf.T.reshape(-1)[:N_PER]
        seg_parts_out.append(seg_core)
        pct = om['pct_o']                        # [96, N_PAD]
        pc_core = pct.reshape(T, 3, N_PAD)[:, :, :N_PER].transpose(0, 2, 1)
        pc_parts.append(pc_core)

    seg_lab = np.concatenate(seg_parts_out).astype(np.int64)
    # map label index back to an actual seg_part value (labels are 0..15 already)
    seg = seg_lab.astype(seg_np.dtype)
    pc_trans = np.concatenate(pc_parts, axis=1).astype(np.float32)
    return pc_trans, seg, trans_list
